# revision 83
# baseline (speedup 1.0000x reference)
"""Trainium2 Bass kernel for a video-diffusion BasicTransformerBlock
(sparse-causal self-attn + cross-attn + GEGLU FF).

Sharding: data-parallel, one (batch, frame) pair per NeuronCore (8 frames ->
8 cores). Each core receives its own frame, frame 0 of its batch, and the
previous frame (duplicated inputs), so the sparse-causal KV gather needs no
collectives. For frames 0/1 the first/former KV frames coincide; softmax over
duplicated keys is mathematically identical to the reference's concat.

On-device layout: activations are feature-major (x^T, [dim, tokens]) so every
projection contracts over SBUF partitions without any transposes. LayerNorm
column-stats come from ones-matmuls; softmax runs max-free (scores bounded ~|7.5|) with denominators from an appended ones-column in V.

v2 perf notes vs v1 (667us -> ~594us measured, rel err 1.3e-2):
- Self-attention AV and both attention out-projections run in fp8e4
  MatmulPerfMode.DoubleRow (2 contraction k-tiles per instruction):
  probs E, V, attn outputs aT and o1/o2 weights are fp8. exp gets a
  constant -2.5 bias so E stays under fp8e4's 240 max normal (cancels
  in the softmax normalization). NOTE: on real TRN2 a dual-fp8 matmul
  measures ~587ns vs ~379ns for a 512-col bf16 matmul (NOT the cost
  model's 0.5 cycles/row), so DR only nets ~20-40%% on these families.
- Everything else on the PE is bf16 (fp32r only for the fp32 residual
  stats): halves weight DMA vs fp32. FF/QK/projections must stay >=bf16:
  fp8 there blows the 2e-2 error budget (measured in numpy: FF fp8 alone
  ~2.3e-2).
- GEGLU gating stays on DVE (gpsimd cannot access PSUM); LN square()
  runs on the otherwise-idle gpsimd engine (SBUF-only).
- Cross-attention K2/V2/enc projections are hoisted into phase A (no
  residual dependence) to fill PE stalls.
- Sustained matmul clock on this part is ~1.35GHz (throttle-limited;
  hw_specs' 2.4GHz PE_CYCLE never materializes for sustained streams),
  so the wall tracks the PE instruction stream almost 1:1.
- Buffer-depth tuning was worth ~50us: PSUM sps ring 2->3 (avps/ps
  shrunk to 1 to stay in 8 banks; cross-attn AV draws from the sps ring
  since its single-matmul groups would serialize on a 1-buf avps),
  weight pool 7->16 bufs (the ring was false-serializing the hoisted
  k2/v2 loads against q1/k1/v1), E pool 4->6, qT 8->11.
- Softmax denominators: DVE reciprocal (not ACT Ln/Exp, which queues
  behind pending exps on the in-order ACT engine) + the batched
  normalize is emitted two groups late so the reciprocal latency hides
  behind the next groups' matmuls.
- k1/v1 loaded once (not per frame); both K projections emitted before
  the V projections so attention QK/exp starts ~25us earlier and V-proj
  matmuls fill its exp waits; hsT_q DMA split by column half so LN1
  stats start before the full frame lands; den ring 4-deep and the
  normalize lag drains to 1 pending near the phase end so the flush
  doesn't bunch in front of the out-projection; o2p weights prefetched
  at phase-D start. Measured: 593us max-of-8-cores / 589.9 mean (pool
  noise +-10us) on the shared pool's good regime.
Numpy-modeled end-to-end error: ~1.2e-2 (budget 2e-2); with USE_O_DR=False
~3.5e-3.

v3 perf notes (594us -> ~520us measured max-of-8 / ~516 mean, rel 1.24e-2):
- Softmax denominators: per-pair chain — DVE copy of the PSUM ones-column
  row to SBUF (the custom-DVE recip misreads PSUM on HW: NaNs), DVE
  reciprocal_approx_fast (0.55us vs 3.4us for InstReciprocal), then a
  gpsimd partition_broadcast (~1us, SBUF->SBUF) replaces the PE
  ones-matmul broadcast. The PE never touches the den chain, so no
  in-order-PE stalls and no p-state resets (stall-adjacent matmuls
  measured ~20%% slower after every gap).
- Same partition_broadcast treatment for LN mean/rstd rows. gpsimd lib is
  pinned to library_config.proxy (tensor_tensor + partition_broadcast in
  ONE lib) — the auto insert_library_loads pass thrash-reloads between
  'standard' and 'attn' otherwise (16us per reload). The one-time load
  still costs ~16us at start, so LN1(q) keeps the old PE-broadcast path
  and its squares go to DVE (pe_bcast/sq_on_dve flags).
- Residual updates (o1/o2/ff2-h0): identity-matmul preloads res into the
  PSUM accumulator, scalar-engine Identity+bias evicts it back — no DVE
  scalar_tensor_tensor on the critical seams. AF.Copy/Identity share
  every ACT table set with exp/ln/gelu (no table reloads).
- o1/LN2 and o2/LN3 interleave by query half: out_proj emits one half,
  the LN stats for that half run while the PE does the other half, and
  q2 consumes per-half (head_proj hh_list) — hides the ~8us LN chains.
- Cross-attn: scores/exp pipelined one PAIR ahead of AV (npairs==1 path);
  PSUM evicts (ub + den row) ride the scalar engine there (DVE was the
  cross-attn throughput limit at ~2.6us/pair).
- aT split into per-head-pair tiles so out_proj pair j depends only on
  its own normalizes, not the whole flush. FF1 first 3 mi staggered
  hh0-first to cover LN3-h1; FF2 h1 drains on DVE while h0 drains on ACT.
- Shared-pool variance is real: occasional runs measure +20..100us slower
  (one 645us outlier on an identical binary); re-measure before
  concluding a change regressed.
"""
import os
import sys
import numpy as np

if not os.environ.get("TRN_TERMINAL_POOL_IPS"):
    raise RuntimeError("expected axon trn environment")
for _p in ("/opt/trn_rl_repo",):
    if _p not in sys.path:
        sys.path.append(_p)

import ml_dtypes
import concourse.bass as bass
import concourse.tile as tile
from concourse import bacc, mybir
from concourse.bass_utils import run_bass_kernel_spmd

FP32 = mybir.dt.float32
F32R = mybir.dt.float32r
BF16 = mybir.dt.bfloat16
FP8 = mybir.dt.float8e4
AF = mybir.ActivationFunctionType
OP = mybir.AluOpType
DR = mybir.MatmulPerfMode.DoubleRow
E4M3 = ml_dtypes.float8_e4m3

D = 640          # model dim
T = 1024         # tokens / frame
H = 8            # heads
DH = 80          # head dim
DKT = D // 128   # 5 feature tiles of the model dim
TT = T // 128    # 8 token tiles / frame
QH = 512         # query half width
CROSS = 768
CKT = CROSS // 128
CTX = 77
CTXP = 80   # context padded for free-dim alignment
DFF = 2560       # ff hidden (per GEGLU half)
FMT = DFF // 128  # 20 ff row tiles per half
LN_EPS = 1e-5
EXP_BIAS = -2.5  # keeps exp(score) under fp8e4 max normal (240)

# bias-pack column offsets ([128, NB] f32)
OB1, OB2, FB2, FBX, FBG = 0, 5, 10, 15, 35
LN_G = {1: 55, 2: 65, 3: 75}
LN_B = {1: 60, 2: 70, 3: 80}
EPS_COL = 85
EXPB_COL = 86
NB = 87

N_CORES = 8
USE_O_DR = True   # fp8 DoubleRow for o1/o2 projections (adds ~8e-3 rel err)

# test hook: CoreSim lacks Gelu; tests may override with a sim-supported func
GELU_AF = None


def r32(ap):
    return ap.bitcast(F32R) if ap.dtype == FP32 else ap


def build_program(ln_trivial):
    nc = bacc.Bacc("TRN2", target_bir_lowering=False, debug=False,
                   num_devices=N_CORES)
    dram = {}
    dram["hsT_q"] = nc.dram_tensor("hsT_q", [D, T], F32R, kind="ExternalInput").ap()
    for name in ("hsT_first", "hsT_former"):
        dram[name] = nc.dram_tensor(name, [D, T], BF16, kind="ExternalInput").ap()
    dram["encT"] = nc.dram_tensor("encT", [CROSS, CTXP], BF16, kind="ExternalInput").ap()
    for name in ("q1", "k1", "v1", "q2"):
        dram[name] = nc.dram_tensor(name, [D, D], BF16, kind="ExternalInput").ap()
    for name in ("k2", "v2"):
        dram[name] = nc.dram_tensor(name, [CROSS, D], BF16, kind="ExternalInput").ap()
    o_dt = FP8 if USE_O_DR else BF16
    for name in ("o1p", "o2p"):
        dram[name] = nc.dram_tensor(name, [128, H * D], o_dt, kind="ExternalInput").ap()
    dram["ff1b"] = nc.dram_tensor("ff1b", [2 * FMT, D, 128], BF16, kind="ExternalInput").ap()
    dram["ff2"] = nc.dram_tensor("ff2", [DFF, D], BF16, kind="ExternalInput").ap()
    dram["biases"] = nc.dram_tensor("biases", [128, NB], FP32, kind="ExternalInput").ap()
    dram["ident"] = nc.dram_tensor("ident", [128, 128], F32R, kind="ExternalInput").ap()
    out_dram = nc.dram_tensor("outT", [D, T], F32R, kind="ExternalOutput").ap()

    scale = float(DH) ** -0.5

    with tile.TileContext(nc) as tc:
        from contextlib import ExitStack
        with ExitStack() as ctx:
            # pin the one gpsimd library that has BOTH tensor_tensor and
            # partition_broadcast — the auto pass thrash-reloads otherwise
            from concourse import library_config
            nc.gpsimd.load_library(library_config.proxy)
            pc = ctx.enter_context(tc.tile_pool(name="const", bufs=1))
            pres = ctx.enter_context(tc.tile_pool(name="res", bufs=5))
            pn = ctx.enter_context(tc.tile_pool(name="n", bufs=6))
            psq = ctx.enter_context(tc.tile_pool(name="sq", bufs=2))
            prow = ctx.enter_context(tc.tile_pool(name="row", bufs=1))
            prcb = ctx.enter_context(tc.tile_pool(name="rcb", bufs=2))
            pw = ctx.enter_context(tc.tile_pool(name="w", bufs=16))
            pps = ctx.enter_context(tc.tile_pool(name="ps", bufs=2, space="PSUM"))

            bias_sb = pc.tile([128, NB], FP32, tag="bias")
            ident_sb = pc.tile([128, 128], F32R, tag="ident")
            invd_f = pc.tile([128, 1], FP32, tag="invdf")
            nc.vector.memset(invd_f[:], 1.0 / D)
            invd = pc.tile([128, 1], F32R, tag="invd")
            nc.vector.tensor_copy(invd[:], invd_f[:])  # fp32r rounding producer
            invd_b = pc.tile([128, 1], BF16, tag="invdb")
            nc.vector.tensor_copy(invd_b[:], invd_f[:])
            onesr_f = pc.tile([128, 128], FP32, tag="onesrf")
            nc.vector.memset(onesr_f[:], 1.0)
            onesr = pc.tile([128, 128], F32R, tag="onesr")
            nc.vector.tensor_copy(onesr[:], onesr_f[:])
            ones_b = pc.tile([128, 128], BF16, tag="onesb")
            nc.vector.tensor_copy(ones_b[:], onesr_f[:])

            def bcol(j):
                return bias_sb[:, j:j + 1]

            def load_w(dname, n_kt, tag, pool, dtype=BF16):
                tiles = []
                for kt in range(n_kt):
                    wt = pool.tile([128, D], dtype, tag=tag, name=f"{dname}_{kt}")
                    nc.sync.dma_start(wt[:], dram[dname][kt * 128:(kt + 1) * 128, :])
                    tiles.append(wt)
                return tiles

            def emit_ln(x_tiles, which, out_tiles, pe_bcast=False,
                        sq_on_dve=False, halves=(0, 1)):
                """Feature-major LN of 5 [128, T] tiles (fp32r or bf16).

                Column stats via ones-matmuls; the mean and rstd rows are
                broadcast across partitions by gpsimd partition_broadcast
                into SBUF (no PE ones-matmul, no PSUM), so the PE stream
                never stalls on the stats chain. pe_bcast keeps the old PE
                ones-matmul broadcast (for LN1(q), which runs before the
                one-time ~16us gpsimd library load finishes); sq_on_dve
                likewise dodges the gpsimd queue for the squares.
                out_tiles: list that receives the 5 result APs (bf16);
                passing x_tiles itself runs the LN in place."""
                in_place = out_tiles is x_tiles
                x_bf = x_tiles[0].dtype == BF16
                inv_l = invd_b if x_bf else invd
                rb_bc = {}
                for hh in halves:
                    sl = slice(hh * QH, (hh + 1) * QH)
                    stp = pps.tile([128, 2 * QH], FP32, tag="sps", bufs=3,
                                   name=f"lnps{which}{hh}")
                    sp = stp[:, 0:QH]
                    spq = stp[:, QH:2 * QH]
                    for kt in range(DKT):
                        nc.tensor.matmul(sp[0:1, :], inv_l[:, 0:1],
                                         x_tiles[kt][:, sl],
                                         start=(kt == 0), stop=(kt == DKT - 1))
                    for kt in range(DKT):
                        sq = psq.tile([128, QH], F32R, tag="sq", name=f"sq{which}{hh}{kt}")
                        sq_eng = nc.vector if sq_on_dve else nc.gpsimd
                        sq_eng.tensor_tensor(sq[:], x_tiles[kt][:, sl],
                                             x_tiles[kt][:, sl], OP.mult)
                        nc.tensor.matmul(spq[0:1, :], invd[:, 0:1], sq[:],
                                         start=(kt == 0), stop=(kt == DKT - 1))
                    # bf16 mu row (same rounding as the old ones-matmul path);
                    # stat evicts ride the scalar engine to keep DVE clear
                    muf = prow.tile([1, QH], BF16, tag="muf", bufs=2,
                                    name=f"muf{which}{hh}")
                    msqf = prow.tile([1, QH], FP32, tag="msqf", bufs=2,
                                     name=f"msqf{which}{hh}")
                    nc.vector.tensor_copy(muf[0:1, :], sp[0:1, :])
                    nc.vector.tensor_copy(msqf[0:1, :], spq[0:1, :])
                    if pe_bcast:
                        mu_bc = pps.tile([128, QH], FP32, tag="avps", bufs=1,
                                         name=f"mub{which}{hh}")
                        nc.tensor.matmul(mu_bc[:, :], ones_b[0:1, :],
                                         muf[0:1, :], start=True, stop=True)
                    else:
                        mu_bc = prow.tile([128, QH], BF16, tag="mubc", bufs=2,
                                          name=f"mubc{which}{hh}")
                        nc.gpsimd.partition_broadcast(mu_bc[:, :], muf[0:1, :])
                    # pass 1: x - mu (from the SBUF broadcast); on the gpsimd
                    # path the subtract runs on gpsimd right behind the
                    # broadcast in the same queue — no cross-engine hop and
                    # no DVE occupancy
                    for kt in range(DKT):
                        if in_place:
                            nt_seg = x_tiles[kt][:, sl]
                        else:
                            if hh == 0:
                                nt = pn.tile([128, T], BF16, tag="n",
                                             name=f"n{which}_{kt}")
                                out_tiles.append(nt)
                            nt_seg = out_tiles[kt][:, sl]
                        nc.vector.tensor_tensor(nt_seg, x_tiles[kt][:, sl],
                                                mu_bc[:, :], OP.subtract)
                    # -var = mu^2 - E[x^2]
                    mup = prow.tile([1, QH], FP32, tag="mup", bufs=2,
                                    name=f"mup{which}{hh}")
                    nc.vector.tensor_tensor(mup[0:1, :], muf[0:1, :],
                                            muf[0:1, :], OP.mult)
                    nc.vector.tensor_tensor(mup[0:1, :], mup[0:1, :],
                                            msqf[0:1, :], OP.subtract)
                    # rstd = exp(-0.5 * ln(var + eps)); ACT Ln/Exp round trip
                    # measured at 1.1e-5 max rel on HW
                    rstdf = prow.tile([1, QH], BF16, tag="rstdf", bufs=2,
                                      name=f"rstdf{which}{hh}")
                    nc.scalar.activation(msqf[0:1, :], mup[0:1, :],
                                         AF.Ln, scale=-1.0,
                                         bias=bias_sb[0:1, EPS_COL:EPS_COL + 1])
                    nc.scalar.activation(rstdf[0:1, :], msqf[0:1, :],
                                         AF.Exp, scale=-0.5)
                    if pe_bcast:
                        rb_bc[hh] = rstdf
                    else:
                        rbc = prow.tile([128, QH], BF16, tag="rbc", bufs=2,
                                        name=f"rbc{which}{hh}")
                        nc.gpsimd.partition_broadcast(rbc[:, :], rstdf[0:1, :])
                        rb_bc[hh] = rbc
                for hh in halves:
                    sl = slice(hh * QH, (hh + 1) * QH)
                    if pe_bcast:
                        rb = pps.tile([128, QH], FP32, tag="avps", bufs=1,
                                      name=f"rb{which}{hh}")
                        nc.tensor.matmul(rb[:, :], ones_b[0:1, :],
                                         rb_bc[hh][0:1, :], start=True,
                                         stop=True)
                    else:
                        rb = rb_bc[hh]
                    for kt in range(DKT):
                        nt_seg = (x_tiles[kt] if in_place else out_tiles[kt])[:, sl]
                        nc.vector.tensor_tensor(nt_seg, nt_seg, rb[:, :],
                                                OP.mult)
                        if not ln_trivial[which - 1]:
                            nc.scalar.activation(nt_seg, nt_seg, AF.Identity,
                                                 bias=bcol(LN_B[which] + kt),
                                                 scale=bcol(LN_G[which] + kt))
                return out_tiles

            def head_proj(w_tiles, n_tiles, out_tiles, col_off, n_kt, tag,
                          hh_list=(0, 1)):
                """out^T[h][0:80, col_off:col_off+T] = w.T @ n, per-head padded.

                With both halves, they share one 2-bank PSUM tile and are
                evicted with a single copy. A single-half call (used to start
                consuming a half-finished LN) evicts just that half."""
                if len(hh_list) == 1:
                    hh = hh_list[0]
                    for h in range(H):
                        qp = pps.tile([128, 2 * QH], FP32, tag="sps", bufs=3,
                                      name=f"hp{tag}{h}{hh}")
                        for kt in range(n_kt):
                            nc.tensor.matmul(
                                qp[0:DH, 0:QH],
                                w_tiles[kt][:, h * DH:(h + 1) * DH],
                                n_tiles[kt][:, hh * QH:(hh + 1) * QH],
                                start=(kt == 0), stop=(kt == n_kt - 1))
                        nc.scalar.copy(
                            out_tiles[h][0:DH,
                                         col_off + hh * QH:col_off + (hh + 1) * QH],
                            qp[0:DH, 0:QH])
                    return
                for h in range(H):
                    qp = pps.tile([128, 2 * QH], FP32, tag="sps", bufs=3,
                                  name=f"hp{tag}{h}")
                    for hh in range(2):
                        for kt in range(n_kt):
                            nc.tensor.matmul(
                                qp[0:DH, hh * QH:(hh + 1) * QH],
                                w_tiles[kt][:, h * DH:(h + 1) * DH],
                                n_tiles[kt][:, hh * QH:(hh + 1) * QH],
                                start=(kt == 0), stop=(kt == n_kt - 1))
                    nc.scalar.copy(
                        out_tiles[h][0:DH, col_off:col_off + T], qp[0:DH, :])

            # V layout: per key-tile-pair blocks of 8 heads x (2 x 112)
            # cols: head h of slot kt lives at (kt//2)*1792 + h*224 +
            # (kt%2)*112, data cols 0:80, ones col 96. The j-pair stride of
            # 112B keeps dual-fp8 ldweights 16B-aligned AND contiguous so
            # the AV DoubleRow weight loads run at full speed.

            def v_heads(v_all, kt, half):
                base = (kt // 2) * 1792 + half * 896
                j = (kt % 2) * 112
                return v_all[:, base:base + 896].rearrange(
                    "p (h c) -> p h c", c=224)[:, :, j:j + 97]

            def v_proj(n_tiles, v_all, kt_slot, n_kt, w_tiles, n_tok, tok_off):
                """token-major V slot: data cols 0:80, ones col at 96 so the
                AV denominator lands on PSUM partition 96 (engine APs must
                start at partition 0/32/64/96)."""
                for half in range(2):
                    hv = v_heads(v_all, kt_slot, half)
                    nc.gpsimd.memset(hv[:, :, 80:96], 0.0)
                    nc.gpsimd.memset(hv[:, :, 96:97], 1.0)
                vpp = pps.tile([128, 2 * QH], FP32, tag="sps", bufs=3, name="vpp")
                for half in range(2):
                    vp = vpp[:, half * QH:half * QH + 320]
                    for kt in range(n_kt):
                        nc.tensor.matmul(
                            vp[0:n_tok, :],
                            n_tiles[kt][:, tok_off:tok_off + n_tok],
                            w_tiles[kt][:, half * 320:(half + 1) * 320],
                            start=(kt == 0), stop=(kt == n_kt - 1))
                    dst = v_heads(v_all, kt_slot, half)[0:n_tok, :, 0:80]
                    src = vp[0:n_tok, :].rearrange("p (h c) -> p h c", c=80)
                    nc.vector.tensor_copy(dst, src)

            def attention(qT_t, kT_t, v_all, n_keytiles, key_dim_last, aT_t,
                          e_pool, av_dr, pre_head=None, lag=2, xpool=None):
                """S^T -> exp -> AV; attention output is evicted unnormalized;
                each pair's denominator gets a fast approx reciprocal (single
                custom-DVE op, ~18 correct bits) straight off the AV PSUM ones
                column, and the normalize is emitted `lag` pairs late so the
                reciprocal latency hides behind the next pairs' matmuls.

                av_dr: E/V are fp8 and AV runs fp8 DoubleRow (2 key tiles
                per matmul at 0.5 cycles/row); exp gets EXP_BIAS so E stays
                under fp8 max normal (cancels in normalization)."""
                pend = {}
                drs = {}
                a_fp8 = aT_t[0].dtype == FP8
                e_dt = FP8 if av_dr else BF16

                def emit_normalize(p):
                    h, hh = p // 2, p % 2
                    rcb = drs.pop(p)
                    seg = aT_t[h][0:DH, hh * QH:(hh + 1) * QH]
                    src_seg = pend.pop(p)[0:DH, :] if a_fp8 else seg
                    nc.vector.tensor_tensor(seg, src_seg, rcb[0:DH, :],
                                            OP.mult)
                npairs = (n_keytiles + 1) // 2
                n_pairs = 2 * H

                def finish_pair(p, avp, evict_on_act=False):
                    """unnormalized evict + per-pair denominator chain.

                    evict_on_act alternates the PSUM reads between the scalar
                    engine (AF.Copy shares the exp table set, no reload) and
                    DVE per pair — cross-attn is throughput-bound on whichever
                    engine takes both, so split the load."""
                    h, hh = p // 2, p % 2
                    ev = nc.scalar.copy if evict_on_act else nc.vector.tensor_copy
                    ev2 = nc.vector.tensor_copy
                    if a_fp8:
                        ub = prcb.tile([128, QH], BF16, tag="ub", bufs=3,
                                       name=f"ub{h}{hh}")
                        pend[p] = ub
                        ev(ub[0:DH, :], avp[0:DH, :])
                    else:
                        ev(aT_t[h][0:DH, hh * QH:(hh + 1) * QH], avp[0:DH, :])
                    # den to SBUF (the custom-DVE recip misreads PSUM on
                    # HW), fast fp32 reciprocal on DVE, then a gpsimd
                    # partition_broadcast into SBUF — the PE never touches
                    # the denominator chain
                    dnf = prcb.tile([1, QH], FP32, tag="dnf", bufs=2,
                                    name=f"dnf{h}{hh}")
                    ev2(dnf[0:1, :], avp[96:97, :])
                    drf = prcb.tile([1, QH], FP32, tag="drf", bufs=2,
                                    name=f"drf{h}{hh}")
                    nc.vector.reciprocal_approx_fast(drf[0:1, :], dnf[0:1, :])
                    rcb = prcb.tile([DH, QH], FP32, tag="dr", bufs=lag + 1,
                                    name=f"dr{h}{hh}")
                    nc.gpsimd.partition_broadcast(rcb[:, :], drf[0:1, :])
                    drs[p] = rcb
                    if p >= lag:
                        emit_normalize(p - lag)

                if npairs == 1 and not av_dr:
                    # single key tile (cross-attn): pipeline scores/exp one
                    # PAIR ahead of AV so the PE never waits on the exp
                    klen = key_dim_last
                    sc = {}
                    for idx in range(n_pairs + 1):
                        if idx < n_pairs:
                            h, hh = idx // 2, idx % 2
                            spp = pps.tile([128, 2 * QH], FP32, tag="sps",
                                           bufs=3, name=f"s{h}{hh}")
                            nc.tensor.matmul(
                                spp[0:klen, 0:QH], kT_t[h][0:DH, 0:klen],
                                qT_t[h][0:DH, hh * QH:(hh + 1) * QH],
                                start=True, stop=True)
                            et = e_pool.tile([128, 2 * QH], e_dt, tag="E",
                                             name=f"e{h}{hh}")
                            nc.scalar.activation(et[0:klen, 0:QH],
                                                 spp[0:klen, 0:QH],
                                                 AF.Exp, scale=scale)
                            sc[idx] = et
                        if idx > 0:
                            p = idx - 1
                            h, hh = p // 2, p % 2
                            pet = sc.pop(p)
                            avp = pps.tile([128, 2 * QH], FP32, tag="sps",
                                           bufs=3, name=f"av{h}{hh}")[:, 0:QH]
                            vh = v_heads(v_all, 0, h // 4)
                            nc.tensor.matmul(avp[0:97, :],
                                             vh[0:klen, h % 4, :],
                                             pet[0:klen, 0:QH],
                                             start=True, stop=True)
                            finish_pair(p, avp, evict_on_act=True)
                    for p in sorted(drs):
                        emit_normalize(p)
                    return

                for h in range(H):
                    if pre_head is not None:
                        pre_head(h)
                    at = aT_t[h]
                    for hh in range(2):
                        p = h * 2 + hh
                        avp = pps.tile([128, QH], FP32, tag="avps", bufs=1,
                                       name=f"av{h}{hh}")
                        # two score tiles share one 2-bank PSUM tile so a
                        # single exp covers both; pipelined one pair ahead of
                        # the AV consumers
                        ets = {}
                        for pt in range(npairs + 1):
                            if pt < npairs:
                                kts = [kt for kt in (2 * pt, 2 * pt + 1)
                                       if kt < n_keytiles]
                                spp = pps.tile([128, 2 * QH], FP32, tag="sps",
                                               bufs=3, name=f"s{h}{hh}{pt}")
                                klens = []
                                for j, kt in enumerate(kts):
                                    klen = (key_dim_last
                                            if kt == n_keytiles - 1 else 128)
                                    klens.append(klen)
                                    nc.tensor.matmul(
                                        spp[0:klen, j * QH:(j + 1) * QH],
                                        kT_t[h][0:DH, kt * 128:kt * 128 + klen],
                                        qT_t[h][0:DH, hh * QH:(hh + 1) * QH],
                                        start=True, stop=True)
                                et = e_pool.tile([128, 2 * QH], e_dt, tag="E",
                                                 name=f"e{h}{hh}{pt}")
                                eb = (dict(bias=bias_sb[0:128, EXPB_COL:EXPB_COL + 1])
                                      if av_dr else {})
                                if len(kts) == 2 and klens[0] == klens[1]:
                                    nc.scalar.activation(
                                        et[0:klens[0], :], spp[0:klens[0], :],
                                        AF.Exp, scale=scale, **eb)
                                else:
                                    for j, kt in enumerate(kts):
                                        nc.scalar.activation(
                                            et[0:klens[j], j * QH:(j + 1) * QH],
                                            spp[0:klens[j], j * QH:(j + 1) * QH],
                                            AF.Exp, scale=scale, **eb)
                                ets[pt] = (et, kts, klens)
                            if pt > 0:
                                pet, pkts, pklens = ets.pop(pt - 1)
                                vb = (pt - 1) * 1792 + h * 224
                                v_ap = v_all[:, vb:vb + 224].rearrange(
                                    "p (j c) -> p j c", c=112)[:, :, 0:97]
                                e_ap = pet[:].rearrange(
                                    "p (j f) -> p j f", f=QH)
                                nc.tensor.matmul(
                                    avp[0:97, :], v_ap, e_ap,
                                    start=(pt == 1), stop=(pt == npairs),
                                    perf_mode=DR)
                        # fp8 aT can't hold unnormalized values (they exceed
                        # 240): finish_pair parks them in bf16 and the lagged
                        # normalize writes the fp8 tile
                        finish_pair(p, avp)
                for p in sorted(drs):
                    emit_normalize(p)

            def out_proj(wp_ap, aT_pairs, res_t, bias_off, hh):
                """res[:, half hh] += aT @ o^T + bias (in-place update).

                Emitted one query-half at a time so the following LN's
                stats can interleave with the other half's matmuls.
                USE_O_DR: fp8 DoubleRow over head pairs (contraction subtile
                = head; weight rows 80:128 of each head slab are zero, which
                also kills the untouched aT padding rows)."""
                sl = slice(hh * QH, (hh + 1) * QH)
                for m in range(DKT):
                    op_ = pps.tile([128, 2 * QH], FP32, tag="sps", bufs=3,
                                   name=f"op{m}{hh}")
                    # preload the residual into PSUM with an identity matmul
                    # so the update needs no DVE pass: the scalar engine
                    # evicts PSUM + bias straight back to res
                    nc.tensor.matmul(op_[:, 0:QH], ident_sb[:, :],
                                     res_t[m][:, sl], start=True, stop=False)
                    if USE_O_DR:
                        for j in range(H // 2):
                            w_ap = wp_ap.rearrange(
                                "p (h c) -> p h c", c=D)[
                                :, 2 * j:2 * j + 2, m * 128:(m + 1) * 128]
                            a_ap = aT_pairs[j].rearrange(
                                "p (h c) -> p h c", c=T)[
                                :, 0:2, sl]
                            nc.tensor.matmul(
                                op_[:, 0:QH], w_ap, a_ap,
                                start=False, stop=(j == H // 2 - 1),
                                perf_mode=DR)
                    else:
                        for kt in range(H):
                            nc.tensor.matmul(
                                op_[:, 0:QH],
                                wp_ap.rearrange("p (h c) -> p h c", c=D)[
                                    :, kt, m * 128:(m + 1) * 128],
                                aT_pairs[kt // 2].rearrange(
                                    "p (h c) -> p h c", c=T)[
                                    :, kt % 2, sl],
                                start=False, stop=(kt == H - 1))
                    nc.scalar.activation(res_t[m][:, sl], op_[:, 0:QH],
                                         AF.Identity,
                                         bias=bcol(bias_off + m))

            # residual stream (feature-major, f32)
            res_tiles = []
            for kt in range(DKT):
                rt = pres.tile([128, T], F32R, tag="res", name=f"res_{kt}")
                res_tiles.append(rt)
            # split by column half, all hh=0 halves first, so the hh=0 LN
            # stats chain starts after half the frame has landed
            for hh in range(2):
                for kt in range(DKT):
                    nc.sync.dma_start(
                        res_tiles[kt][:, hh * QH:(hh + 1) * QH],
                        dram["hsT_q"][kt * 128:(kt + 1) * 128,
                                      hh * QH:(hh + 1) * QH])
            # constants ride behind the residual halves (first needed ~15us in)
            nc.sync.dma_start(bias_sb[:], dram["biases"][:])
            nc.sync.dma_start(ident_sb[:], dram["ident"][:])

            a_dt = FP8 if USE_O_DR else BF16

            with ExitStack() as ctx_abcd:
                pqT = ctx_abcd.enter_context(tc.tile_pool(name="qT", bufs=11))
                paT = ctx_abcd.enter_context(tc.tile_pool(name="aT", bufs=8))

                def alloc_aT(nm):
                    """per-head-pair aT tiles: out_proj's pair j then depends
                    only on its own pairs' normalizes, not the whole flush"""
                    pairs = [paT.tile([128, 2 * T], a_dt, tag="aT",
                                      name=f"{nm}_{j}") for j in range(4)]
                    for pt in pairs:
                        nc.gpsimd.memset(
                            pt[64:128, :].bitcast(FP32)
                            if a_dt == FP8 else pt[64:128, :], 0.0)
                    tiles = [pairs[h // 2][:, (h % 2) * T:(h % 2 + 1) * T]
                             for h in range(H)]
                    return pairs, tiles
                penc = ctx_abcd.enter_context(tc.tile_pool(name="enc", bufs=6))
                pk2 = ctx_abcd.enter_context(tc.tile_pool(name="k2T", bufs=8))
                pV2 = ctx_abcd.enter_context(tc.tile_pool(name="V2", bufs=1))

                # ---------- phase A: LN1 + QKV projections ----------
                with ExitStack() as ctx_b:
                    pkT = ctx_b.enter_context(tc.tile_pool(name="kT", bufs=8))
                    pV = ctx_b.enter_context(tc.tile_pool(name="V", bufs=1))
                    pE = ctx_b.enter_context(tc.tile_pool(name="E", bufs=5))

                    kT_tiles = [pkT.tile([128, 2 * T], BF16, tag="kT", name=f"kT_{h}")
                                for h in range(H)]
                    v_all = pV.tile([128, TT * 1792], FP8, tag="V", name="v_all")

                    fr0_tiles = []
                    fr1_tiles = []
                    for kt in range(DKT):
                        ft = pn.tile([128, T], BF16, tag="fr", bufs=10,
                                     name=f"fr0_{kt}")
                        nc.sync.dma_start(
                            ft[:], dram["hsT_first"][kt * 128:(kt + 1) * 128, :])
                        fr0_tiles.append(ft)
                        ft1 = pn.tile([128, T], BF16, tag="fr", bufs=10,
                                      name=f"fr1_{kt}")
                        nc.sync.dma_start(
                            ft1[:], dram["hsT_former"][kt * 128:(kt + 1) * 128, :])
                        fr1_tiles.append(ft1)
                    n_q = emit_ln(res_tiles, 1, [], pe_bcast=True,
                                  sq_on_dve=True)
                    emit_ln(fr0_tiles, 1, fr0_tiles,  # in place, overlaps Q
                            sq_on_dve=True)
                    q1_sb = load_w("q1", DKT, "w", pw)
                    qT_tiles = [pqT.tile([128, T], BF16, tag="qT", name=f"qT_{h}")
                                for h in range(H)]
                    head_proj(q1_sb, n_q, qT_tiles, 0, DKT, "q")

                    # cross-attention K/V from the text context have no
                    # dependence on the residual stream: emit them here so
                    # their PE/DVE work fills phase A/B stalls and phase D
                    # starts with K2/V2 ready.
                    enc_tiles = []
                    for kt in range(CKT):
                        et_ = penc.tile([128, CTXP], BF16, tag="enc", name=f"enc_{kt}")
                        nc.sync.dma_start(
                            et_[:], dram["encT"][kt * 128:(kt + 1) * 128, :])
                        enc_tiles.append(et_)
                    k2_sb = load_w("k2", CKT, "w", pw)
                    k2T_tiles = [pk2.tile([128, CTXP], BF16, tag="k2T", name=f"k2T_{h}")
                                 for h in range(H)]
                    for h in range(H):
                        kp = pps.tile([128, CTXP], FP32, tag="ps", bufs=1, name=f"k2p{h}")
                        for kt in range(CKT):
                            nc.tensor.matmul(kp[0:DH, :],
                                             k2_sb[kt][:, h * DH:(h + 1) * DH],
                                             enc_tiles[kt][:],
                                             start=(kt == 0), stop=(kt == CKT - 1))
                        nc.vector.tensor_copy(k2T_tiles[h][0:DH, :], kp[0:DH, :])
                    v2_sb = load_w("v2", CKT, "w", pw)
                    v2_all = pV2.tile([128, 1792], BF16, tag="V2", name="v2t")
                    v_proj(enc_tiles, v2_all, 0, CKT, v2_sb, CTX, 0)

                    emit_ln(fr1_tiles, 1, fr1_tiles, sq_on_dve=True)  # in place
                    k1_sb = load_w("k1", DKT, "w", pw)
                    frames = (fr0_tiles, fr1_tiles)
                    for fi, fr_tiles in enumerate(frames):
                        head_proj(k1_sb, fr_tiles, kT_tiles, fi * T, DKT, f"k{fi}")
                    v1_sb = load_w("v1", DKT, "w", pw)
                    for fi, fr_tiles in enumerate(frames):
                        for tt in range(TT):
                            v_proj(fr_tiles, v_all, fi * TT + tt, DKT, v1_sb,
                                   128, tt * 128)

                    # ---------- phase B: sparse-causal attention ----------
                    aT_pairs, aT_tiles = alloc_aT("aT")
                    attention(qT_tiles, kT_tiles, v_all, 2 * TT, 128, aT_tiles,
                              pE, av_dr=True)

                # ---------- phase C/D: o1 + LN2 interleaved by half, then
                # cross attention, then o2 + LN3 interleaved by half ----------
                with ExitStack() as ctx_d:
                    pwp = ctx_d.enter_context(tc.tile_pool(name="wp", bufs=1))
                    o1p_sb = pwp.tile([128, H * D], a_dt, tag="wp", name="o1p_sb")
                    # split by head-pair slab so the first DR matmul starts
                    # after a quarter of the weights have landed
                    for j in range(4):
                        nc.sync.dma_start(
                            o1p_sb[:, j * 2 * D:(j + 1) * 2 * D],
                            dram["o1p"][:, j * 2 * D:(j + 1) * 2 * D])
                    pE2 = ctx_d.enter_context(tc.tile_pool(name="E2", bufs=5))
                    pxat = ctx_d.enter_context(tc.tile_pool(name="xat", bufs=2))
                    pwp2 = ctx_d.enter_context(tc.tile_pool(name="wp2", bufs=1))
                    # o2 weights fetched early so the DMA overlaps
                    # LN2/q2/attention instead of stalling the o2 seam
                    o2p_sb = pwp2.tile([128, H * D], a_dt, tag="wp2", name="o2p_sb")
                    nc.sync.dma_start(o2p_sb[:], dram["o2p"][:])
                    q2_sb = load_w("q2", DKT, "w", pw)

                    # o1 half h feeds LN2 half h stats while the PE runs the
                    # other half's matmuls — the LN chain latency hides
                    n2 = []
                    out_proj(o1p_sb[:], aT_pairs, res_tiles, OB1, 0)
                    emit_ln(res_tiles, 2, n2, halves=(0,))
                    out_proj(o1p_sb[:], aT_pairs, res_tiles, OB1, 1)
                    emit_ln(res_tiles, 2, n2, halves=(1,))
                    q2T_tiles = [pqT.tile([128, T], BF16, tag="qT", name=f"q2T_{h}")
                                 for h in range(H)]
                    head_proj(q2_sb, n2, q2T_tiles, 0, DKT, "q2", hh_list=(0,))
                    head_proj(q2_sb, n2, q2T_tiles, 0, DKT, "q2", hh_list=(1,))

                    a2T_pairs, a2T_tiles = alloc_aT("a2T")
                    attention(q2T_tiles, k2T_tiles, v2_all, 1, CTX, a2T_tiles,
                              pE2, av_dr=False, xpool=pxat)
                    n3 = []
                    out_proj(o2p_sb[:], a2T_pairs, res_tiles, OB2, 0)
                    emit_ln(res_tiles, 3, n3, halves=(0,))
                    out_proj(o2p_sb[:], a2T_pairs, res_tiles, OB2, 1)
                    emit_ln(res_tiles, 3, n3, halves=(1,))

            # ---------- phase E: GEGLU feed-forward ----------
            with ExitStack() as ctx_e:
                pG = ctx_e.enter_context(tc.tile_pool(name="gT", bufs=20))
                pgl = ctx_e.enter_context(tc.tile_pool(name="gl", bufs=3))
                pff2 = ctx_e.enter_context(tc.tile_pool(name="ff2w", bufs=20))

                gT_tiles = []
                fxg = {}

                def ff1_tile(mi):
                    fx = pw.tile([128, D], BF16, tag="w", name=f"fx{mi}")
                    fg = pw.tile([128, D], BF16, tag="w", name=f"fg{mi}")
                    fx_dst = fx[:].rearrange("p (k c) -> p k c", c=128)
                    fg_dst = fg[:].rearrange("p (k c) -> p k c", c=128)
                    fx_src = dram["ff1b"][mi].rearrange("(k p) c -> p k c", p=128)
                    fg_src = dram["ff1b"][FMT + mi].rearrange("(k p) c -> p k c", p=128)
                    nc.sync.dma_start(fx_dst, fx_src)
                    nc.sync.dma_start(fg_dst, fg_src)
                    fxg[mi] = (fx, fg)
                    gt = pG.tile([128, T], BF16, tag="gT", name=f"gT_{mi}")
                    gT_tiles.append(gt)

                # the first tiles run their hh=0 half only, so LN3's hh=1
                # chain drains while the PE is already projecting
                STAG = 5
                seq = [(mi, 0) for mi in range(STAG)]
                seq += [(mi, 1) for mi in range(STAG)]
                seq += [(mi, hh) for mi in range(STAG, FMT) for hh in range(2)]
                for mi, hh in seq:
                    if hh == 0:
                        if mi not in fxg:
                            ff1_tile(mi)
                    if True:
                        fx, fg = fxg[mi]
                        gt = gT_tiles[mi]
                        xgp = pps.tile([128, 2 * QH], FP32, tag="sps", bufs=3,
                                       name=f"xgp{mi}{hh}")
                        xp = xgp[:, 0:QH]
                        gp = xgp[:, QH:2 * QH]
                        for kt in range(DKT):
                            nc.tensor.matmul(
                                xp[:, :], fx[:, kt * 128:(kt + 1) * 128],
                                n3[kt][:, hh * QH:(hh + 1) * QH],
                                start=(kt == 0), stop=(kt == DKT - 1))
                        for kt in range(DKT):
                            nc.tensor.matmul(
                                gp[:, :], fg[:, kt * 128:(kt + 1) * 128],
                                n3[kt][:, hh * QH:(hh + 1) * QH],
                                start=(kt == 0), stop=(kt == DKT - 1))
                        gl = pgl.tile([128, QH], BF16, tag="gl", name=f"gl{mi}{hh}")
                        nc.scalar.activation(gl[:], gp[:, :], GELU_AF or AF.Gelu,
                                             bias=bcol(FBG + mi), scale=1.0)
                        nc.vector.scalar_tensor_tensor(
                            gt[:, hh * QH:(hh + 1) * QH], xp[:, :], bcol(FBX + mi),
                            gl[:], OP.add, OP.mult)

                ff2_sb = load_w("ff2", FMT, "ff2w", pff2, dtype=BF16)
                for hh in range(2):
                    sl = slice(hh * QH, (hh + 1) * QH)
                    for m in range(DKT):
                        fp = pps.tile([128, 2 * QH], FP32, tag="sps", bufs=3,
                                      name=f"fp{m}{hh}")
                        if hh == 0:
                            nc.tensor.matmul(fp[:, 0:QH], ident_sb[:, :],
                                             res_tiles[m][:, sl], start=True,
                                             stop=False)
                        for kt in range(FMT):
                            nc.tensor.matmul(
                                fp[:, 0:QH],
                                ff2_sb[kt][:, m * 128:(m + 1) * 128],
                                gT_tiles[kt][:, sl],
                                start=(hh == 1 and kt == 0),
                                stop=(kt == FMT - 1))
                        if hh == 0:
                            nc.scalar.activation(res_tiles[m][:, sl],
                                                 fp[:, 0:QH], AF.Identity,
                                                 bias=bcol(FB2 + m))
                        else:
                            # second half drains on DVE (idle at the end) so
                            # ACT and DVE evict the tail in parallel
                            nc.vector.scalar_tensor_tensor(
                                res_tiles[m][:, sl], fp[:, 0:QH],
                                bcol(FB2 + m), res_tiles[m][:, sl],
                                OP.add, OP.add)
                        nc.sync.dma_start(
                            out_dram[m * 128:(m + 1) * 128, sl],
                            res_tiles[m][:, sl])

    nc.compile()
    return nc


def _install_ntff_shim():
    """Register the axon NTFF profile hook (profiling only; this container's
    antenv lacks the axon_hooks shim module)."""
    import types
    if "antenv.axon_hooks" in sys.modules:
        return
    mod = types.ModuleType("antenv.axon_hooks")
    mod._hook = None
    mod.set_axon_ntff_profile_hook = lambda h: setattr(mod, "_hook", h)
    mod.get_axon_ntff_profile_hook = lambda: mod._hook
    sys.modules["antenv.axon_hooks"] = mod
    try:
        from trn_agent_boot.trn_boot import _ntff_profile_via_ctypes
        mod._hook = _ntff_profile_via_ctypes("/opt/axon/libaxon_pjrt.so")
    except Exception:
        pass


_PROGRAM_CACHE = {}


def _get_program(ln_trivial):
    key = (tuple(ln_trivial), GELU_AF)
    if key not in _PROGRAM_CACHE:
        _PROGRAM_CACHE[key] = build_program(ln_trivial)
    return _PROGRAM_CACHE[key]


def _pad_heads(w):
    """[640, 640] head rows -> [128, 8*640] per-head slabs (rows 0:80)."""
    out = np.zeros((128, H * D), np.float32)
    for h in range(H):
        out[:DH, h * D:(h + 1) * D] = w[h * DH:(h + 1) * DH]
    return out


def _bias_cols(vec, n):
    return np.ascontiguousarray(vec.reshape(n, 128).T)


def kernel(**inputs):
    hs = np.ascontiguousarray(inputs["hidden_states"], np.float32)
    enc = np.ascontiguousarray(inputs["encoder_hidden_states"], np.float32)
    f = int(inputs["video_length"])
    BF = hs.shape[0]
    assert BF == N_CORES and hs.shape[1:] == (T, D)

    ln_trivial = tuple(
        bool(np.all(inputs[f"n{i}_g"] == 1.0) and np.all(inputs[f"n{i}_b"] == 0.0))
        for i in (1, 2, 3))
    nc = _get_program(ln_trivial)

    biases = np.zeros((128, NB), np.float32)
    biases[:, EPS_COL] = LN_EPS
    biases[:, EXPB_COL] = EXP_BIAS
    biases[:, OB1:OB1 + 5] = _bias_cols(inputs["o1_b"].astype(np.float32), 5)
    biases[:, OB2:OB2 + 5] = _bias_cols(inputs["o2_b"].astype(np.float32), 5)
    biases[:, FB2:FB2 + 5] = _bias_cols(inputs["ff2_b"].astype(np.float32), 5)
    ff1_b = inputs["ff1_b"].astype(np.float32)
    biases[:, FBX:FBX + FMT] = _bias_cols(ff1_b[:DFF], FMT)
    biases[:, FBG:FBG + FMT] = _bias_cols(ff1_b[DFF:], FMT)
    for i in (1, 2, 3):
        biases[:, LN_G[i]:LN_G[i] + 5] = _bias_cols(inputs[f"n{i}_g"].astype(np.float32), 5)
        biases[:, LN_B[i]:LN_B[i] + 5] = _bias_cols(inputs[f"n{i}_b"].astype(np.float32), 5)

    ff1 = inputs["ff1"].astype(np.float32)  # [640, 5120]
    ff1b = np.ascontiguousarray(
        ff1.reshape(DKT, 128, 2 * FMT, 128).transpose(2, 0, 1, 3)
        .reshape(2 * FMT, D, 128)).astype(ml_dtypes.bfloat16)

    o_np = E4M3 if USE_O_DR else ml_dtypes.bfloat16
    common = {
        "q1": inputs["q1"].astype(ml_dtypes.bfloat16),
        "k1": inputs["k1"].astype(ml_dtypes.bfloat16),
        "v1": inputs["v1"].astype(ml_dtypes.bfloat16),
        "q2": inputs["q2"].astype(ml_dtypes.bfloat16),
        "k2": inputs["k2"].astype(ml_dtypes.bfloat16),
        "v2": inputs["v2"].astype(ml_dtypes.bfloat16),
        "o1p": _pad_heads(inputs["o1"].astype(np.float32)).astype(o_np),
        "o2p": _pad_heads(inputs["o2"].astype(np.float32)).astype(o_np),
        "ff1b": ff1b,
        "ff2": np.ascontiguousarray(inputs["ff2"], np.float32).astype(ml_dtypes.bfloat16),
        "biases": biases,
        "ident": np.eye(128, dtype=np.float32),
    }

    hsT = np.ascontiguousarray(hs.transpose(0, 2, 1))      # [BF, 640, 1024]
    hsT_bf = hsT.astype(ml_dtypes.bfloat16)
    encT = np.zeros((BF, CROSS, CTXP), np.float32)         # ctx padded 77 -> 80
    encT[:, :, :CTX] = enc.transpose(0, 2, 1)
    encT_bf = encT.astype(ml_dtypes.bfloat16)
    in_maps = []
    for g in range(BF):
        bi, fi = divmod(g, f)
        first = bi * f
        former = bi * f + max(fi - 1, 0)
        in_maps.append({
            **common,
            "hsT_q": hsT[g],
            "hsT_first": hsT_bf[first],
            "hsT_former": hsT_bf[former],
            "encT": encT_bf[g],
        })

    want_trace = bool(int(os.environ.get("KERNEL_TRACE", "0")))
    if want_trace:
        _install_ntff_shim()
    res = run_bass_kernel_spmd(nc, in_maps, core_ids=list(range(N_CORES)),
                               trace=want_trace)
    kernel.last_results = res
    out = np.stack([res.results[g]["outT"].T for g in range(BF)])
    return np.ascontiguousarray(out.astype(inputs["hidden_states"].dtype))



# revision 90
# speedup vs baseline: 1.0012x; 1.0012x over previous
"""Trainium2 Bass kernel for a video-diffusion BasicTransformerBlock
(sparse-causal self-attn + cross-attn + GEGLU FF).

Sharding: data-parallel, one (batch, frame) pair per NeuronCore (8 frames ->
8 cores). Each core receives its own frame, frame 0 of its batch, and the
previous frame (duplicated inputs), so the sparse-causal KV gather needs no
collectives. For frames 0/1 the first/former KV frames coincide; softmax over
duplicated keys is mathematically identical to the reference's concat.

On-device layout: activations are feature-major (x^T, [dim, tokens]) so every
projection contracts over SBUF partitions without any transposes. LayerNorm
column-stats come from ones-matmuls; softmax runs max-free (scores bounded ~|7.5|) with denominators from an appended ones-column in V.

v2 perf notes vs v1 (667us -> ~594us measured, rel err 1.3e-2):
- Self-attention AV and both attention out-projections run in fp8e4
  MatmulPerfMode.DoubleRow (2 contraction k-tiles per instruction):
  probs E, V, attn outputs aT and o1/o2 weights are fp8. exp gets a
  constant -2.5 bias so E stays under fp8e4's 240 max normal (cancels
  in the softmax normalization). NOTE: on real TRN2 a dual-fp8 matmul
  measures ~587ns vs ~379ns for a 512-col bf16 matmul (NOT the cost
  model's 0.5 cycles/row), so DR only nets ~20-40%% on these families.
- Everything else on the PE is bf16 (fp32r only for the fp32 residual
  stats): halves weight DMA vs fp32. FF/QK/projections must stay >=bf16:
  fp8 there blows the 2e-2 error budget (measured in numpy: FF fp8 alone
  ~2.3e-2).
- GEGLU gating stays on DVE (gpsimd cannot access PSUM); LN square()
  runs on the otherwise-idle gpsimd engine (SBUF-only).
- Cross-attention K2/V2/enc projections are hoisted into phase A (no
  residual dependence) to fill PE stalls.
- Sustained matmul clock on this part is ~1.35GHz (throttle-limited;
  hw_specs' 2.4GHz PE_CYCLE never materializes for sustained streams),
  so the wall tracks the PE instruction stream almost 1:1.
- Buffer-depth tuning was worth ~50us: PSUM sps ring 2->3 (avps/ps
  shrunk to 1 to stay in 8 banks; cross-attn AV draws from the sps ring
  since its single-matmul groups would serialize on a 1-buf avps),
  weight pool 7->16 bufs (the ring was false-serializing the hoisted
  k2/v2 loads against q1/k1/v1), E pool 4->6, qT 8->11.
- Softmax denominators: DVE reciprocal (not ACT Ln/Exp, which queues
  behind pending exps on the in-order ACT engine) + the batched
  normalize is emitted two groups late so the reciprocal latency hides
  behind the next groups' matmuls.
- k1/v1 loaded once (not per frame); both K projections emitted before
  the V projections so attention QK/exp starts ~25us earlier and V-proj
  matmuls fill its exp waits; hsT_q DMA split by column half so LN1
  stats start before the full frame lands; den ring 4-deep and the
  normalize lag drains to 1 pending near the phase end so the flush
  doesn't bunch in front of the out-projection; o2p weights prefetched
  at phase-D start. Measured: 593us max-of-8-cores / 589.9 mean (pool
  noise +-10us) on the shared pool's good regime.
Numpy-modeled end-to-end error: ~1.2e-2 (budget 2e-2); with USE_O_DR=False
~3.5e-3.

v3 perf notes (594us -> ~510us measured max-of-8 / ~506 mean, rel 1.24e-2):
- Softmax denominators: per-pair chain — DVE copy of the PSUM ones-column
  row to SBUF (the custom-DVE recip misreads PSUM on HW: NaNs), DVE
  reciprocal_approx_fast (0.55us vs 3.4us for InstReciprocal), then a
  gpsimd partition_broadcast (~1us, SBUF->SBUF) replaces the PE
  ones-matmul broadcast. The PE never touches the den chain, so no
  in-order-PE stalls and no p-state resets (stall-adjacent matmuls
  measured ~20%% slower after every gap).
- Same partition_broadcast treatment for LN mean/rstd rows. gpsimd lib is
  pinned to library_config.proxy (tensor_tensor + partition_broadcast in
  ONE lib) — the auto insert_library_loads pass thrash-reloads between
  'standard' and 'attn' otherwise (16us per reload). The one-time load
  still costs ~16us at start, so LN1(q) keeps the old PE-broadcast path
  and its squares go to DVE (pe_bcast/sq_on_dve flags).
- Residual updates (o1/o2/ff2-h0): identity-matmul preloads res into the
  PSUM accumulator, scalar-engine Identity+bias evicts it back — no DVE
  scalar_tensor_tensor on the critical seams. AF.Copy/Identity share
  every ACT table set with exp/ln/gelu (no table reloads).
- o1/LN2 and o2/LN3 interleave by query half: out_proj emits one half,
  the LN stats for that half run while the PE does the other half, and
  q2 consumes per-half (head_proj hh_list) — hides the ~8us LN chains.
- Cross-attn: scores/exp pipelined one PAIR ahead of AV (npairs==1 path);
  PSUM evicts (ub + den row) ride the scalar engine there (DVE was the
  cross-attn throughput limit at ~2.6us/pair).
- aT split into per-head-pair tiles so out_proj pair j depends only on
  its own normalizes, not the whole flush. FF1 first 3 mi staggered
  hh0-first to cover LN3-h1; FF2 h1 drains on DVE while h0 drains on ACT.
- Tried and reverted: per-HEAD 1024-wide cross-attn chains (PSUM banks
  cap matmuls at 512 fp32 cols; the 1024-wide DVE normalize made DVE
  the limiter, ~+5us), sq/sub passes on gpsimd or ACT at the LN2/LN3
  seams (overloads the exact window the interleave needs).
- Shared-pool variance is real: occasional runs measure +20..100us slower
  (one 645us outlier on an identical binary); re-measure before
  concluding a change regressed.
"""
import os
import sys
import numpy as np

if not os.environ.get("TRN_TERMINAL_POOL_IPS"):
    raise RuntimeError("expected axon trn environment")
for _p in ("/opt/trn_rl_repo",):
    if _p not in sys.path:
        sys.path.append(_p)

import ml_dtypes
import concourse.bass as bass
import concourse.tile as tile
from concourse import bacc, mybir
from concourse.bass_utils import run_bass_kernel_spmd

FP32 = mybir.dt.float32
F32R = mybir.dt.float32r
BF16 = mybir.dt.bfloat16
FP8 = mybir.dt.float8e4
AF = mybir.ActivationFunctionType
OP = mybir.AluOpType
DR = mybir.MatmulPerfMode.DoubleRow
E4M3 = ml_dtypes.float8_e4m3

D = 640          # model dim
T = 1024         # tokens / frame
H = 8            # heads
DH = 80          # head dim
DKT = D // 128   # 5 feature tiles of the model dim
TT = T // 128    # 8 token tiles / frame
QH = 512         # query half width
CROSS = 768
CKT = CROSS // 128
CTX = 77
CTXP = 80   # context padded for free-dim alignment
DFF = 2560       # ff hidden (per GEGLU half)
FMT = DFF // 128  # 20 ff row tiles per half
LN_EPS = 1e-5
EXP_BIAS = -2.5  # keeps exp(score) under fp8e4 max normal (240)

# bias-pack column offsets ([128, NB] f32)
OB1, OB2, FB2, FBX, FBG = 0, 5, 10, 15, 35
LN_G = {1: 55, 2: 65, 3: 75}
LN_B = {1: 60, 2: 70, 3: 80}
EPS_COL = 85
EXPB_COL = 86
NB = 87

N_CORES = 8
USE_O_DR = True   # fp8 DoubleRow for o1/o2 projections (adds ~8e-3 rel err)

# test hook: CoreSim lacks Gelu; tests may override with a sim-supported func
GELU_AF = None


def r32(ap):
    return ap.bitcast(F32R) if ap.dtype == FP32 else ap


def build_program(ln_trivial):
    nc = bacc.Bacc("TRN2", target_bir_lowering=False, debug=False,
                   num_devices=N_CORES)
    dram = {}
    dram["hsT_q"] = nc.dram_tensor("hsT_q", [D, T], F32R, kind="ExternalInput").ap()
    for name in ("hsT_first", "hsT_former"):
        dram[name] = nc.dram_tensor(name, [D, T], BF16, kind="ExternalInput").ap()
    dram["encT"] = nc.dram_tensor("encT", [CROSS, CTXP], BF16, kind="ExternalInput").ap()
    for name in ("q1", "k1", "v1", "q2"):
        dram[name] = nc.dram_tensor(name, [D, D], BF16, kind="ExternalInput").ap()
    for name in ("k2", "v2"):
        dram[name] = nc.dram_tensor(name, [CROSS, D], BF16, kind="ExternalInput").ap()
    o_dt = FP8 if USE_O_DR else BF16
    for name in ("o1p", "o2p"):
        dram[name] = nc.dram_tensor(name, [128, H * D], o_dt, kind="ExternalInput").ap()
    dram["ff1b"] = nc.dram_tensor("ff1b", [2 * FMT, D, 128], BF16, kind="ExternalInput").ap()
    dram["ff2"] = nc.dram_tensor("ff2", [DFF, D], BF16, kind="ExternalInput").ap()
    dram["biases"] = nc.dram_tensor("biases", [128, NB], FP32, kind="ExternalInput").ap()
    dram["ident"] = nc.dram_tensor("ident", [128, 128], F32R, kind="ExternalInput").ap()
    out_dram = nc.dram_tensor("outT", [D, T], F32R, kind="ExternalOutput").ap()

    scale = float(DH) ** -0.5

    with tile.TileContext(nc) as tc:
        from contextlib import ExitStack
        with ExitStack() as ctx:
            # pin the one gpsimd library that has BOTH tensor_tensor and
            # partition_broadcast — the auto pass thrash-reloads otherwise
            from concourse import library_config
            nc.gpsimd.load_library(library_config.proxy)
            pc = ctx.enter_context(tc.tile_pool(name="const", bufs=1))
            pres = ctx.enter_context(tc.tile_pool(name="res", bufs=5))
            pn = ctx.enter_context(tc.tile_pool(name="n", bufs=6))
            psq = ctx.enter_context(tc.tile_pool(name="sq", bufs=2))
            prow = ctx.enter_context(tc.tile_pool(name="row", bufs=1))
            prcb = ctx.enter_context(tc.tile_pool(name="rcb", bufs=2))
            pw = ctx.enter_context(tc.tile_pool(name="w", bufs=16))
            pps = ctx.enter_context(tc.tile_pool(name="ps", bufs=2, space="PSUM"))

            bias_sb = pc.tile([128, NB], FP32, tag="bias")
            ident_sb = pc.tile([128, 128], F32R, tag="ident")
            invd_f = pc.tile([128, 1], FP32, tag="invdf")
            nc.vector.memset(invd_f[:], 1.0 / D)
            invd = pc.tile([128, 1], F32R, tag="invd")
            nc.vector.tensor_copy(invd[:], invd_f[:])  # fp32r rounding producer
            invd_b = pc.tile([128, 1], BF16, tag="invdb")
            nc.vector.tensor_copy(invd_b[:], invd_f[:])
            onesr_f = pc.tile([128, 128], FP32, tag="onesrf")
            nc.vector.memset(onesr_f[:], 1.0)
            onesr = pc.tile([128, 128], F32R, tag="onesr")
            nc.vector.tensor_copy(onesr[:], onesr_f[:])
            ones_b = pc.tile([128, 128], BF16, tag="onesb")
            nc.vector.tensor_copy(ones_b[:], onesr_f[:])

            def bcol(j):
                return bias_sb[:, j:j + 1]

            def load_w(dname, n_kt, tag, pool, dtype=BF16):
                tiles = []
                for kt in range(n_kt):
                    wt = pool.tile([128, D], dtype, tag=tag, name=f"{dname}_{kt}")
                    nc.sync.dma_start(wt[:], dram[dname][kt * 128:(kt + 1) * 128, :])
                    tiles.append(wt)
                return tiles

            def emit_ln(x_tiles, which, out_tiles, pe_bcast=False,
                        sq_on_dve=False, halves=(0, 1)):
                """Feature-major LN of 5 [128, T] tiles (fp32r or bf16).

                Column stats via ones-matmuls; the mean and rstd rows are
                broadcast across partitions by gpsimd partition_broadcast
                into SBUF (no PE ones-matmul, no PSUM), so the PE stream
                never stalls on the stats chain. pe_bcast keeps the old PE
                ones-matmul broadcast (for LN1(q), which runs before the
                one-time ~16us gpsimd library load finishes); sq_on_dve
                likewise dodges the gpsimd queue for the squares.
                out_tiles: list that receives the 5 result APs (bf16);
                passing x_tiles itself runs the LN in place."""
                in_place = out_tiles is x_tiles
                x_bf = x_tiles[0].dtype == BF16
                inv_l = invd_b if x_bf else invd
                rb_bc = {}
                for hh in halves:
                    sl = slice(hh * QH, (hh + 1) * QH)
                    stp = pps.tile([128, 2 * QH], FP32, tag="sps", bufs=3,
                                   name=f"lnps{which}{hh}")
                    sp = stp[:, 0:QH]
                    spq = stp[:, QH:2 * QH]
                    for kt in range(DKT):
                        nc.tensor.matmul(sp[0:1, :], inv_l[:, 0:1],
                                         x_tiles[kt][:, sl],
                                         start=(kt == 0), stop=(kt == DKT - 1))
                    for kt in range(DKT):
                        sq = psq.tile([128, QH], F32R, tag="sq", name=f"sq{which}{hh}{kt}")
                        sq_eng = nc.vector if sq_on_dve else nc.gpsimd
                        sq_eng.tensor_tensor(sq[:], x_tiles[kt][:, sl],
                                             x_tiles[kt][:, sl], OP.mult)
                        nc.tensor.matmul(spq[0:1, :], invd[:, 0:1], sq[:],
                                         start=(kt == 0), stop=(kt == DKT - 1))
                    # bf16 mu row (same rounding as the old ones-matmul path);
                    # stat evicts ride the scalar engine to keep DVE clear
                    muf = prow.tile([1, QH], BF16, tag="muf", bufs=2,
                                    name=f"muf{which}{hh}")
                    msqf = prow.tile([1, QH], FP32, tag="msqf", bufs=2,
                                     name=f"msqf{which}{hh}")
                    nc.vector.tensor_copy(muf[0:1, :], sp[0:1, :])
                    nc.vector.tensor_copy(msqf[0:1, :], spq[0:1, :])
                    if pe_bcast:
                        mu_bc = pps.tile([128, QH], FP32, tag="avps", bufs=2,
                                         name=f"mub{which}{hh}")
                        nc.tensor.matmul(mu_bc[:, :], ones_b[0:1, :],
                                         muf[0:1, :], start=True, stop=True)
                    else:
                        mu_bc = prow.tile([128, QH], BF16, tag="mubc", bufs=2,
                                          name=f"mubc{which}{hh}")
                        nc.gpsimd.partition_broadcast(mu_bc[:, :], muf[0:1, :])
                    # pass 1: x - mu (from the SBUF broadcast); on the gpsimd
                    # path the subtract runs on gpsimd right behind the
                    # broadcast in the same queue — no cross-engine hop and
                    # no DVE occupancy
                    for kt in range(DKT):
                        if in_place:
                            nt_seg = x_tiles[kt][:, sl]
                        else:
                            if hh == 0:
                                nt = pn.tile([128, T], BF16, tag="n",
                                             name=f"n{which}_{kt}")
                                out_tiles.append(nt)
                            nt_seg = out_tiles[kt][:, sl]
                        nc.vector.tensor_tensor(nt_seg, x_tiles[kt][:, sl],
                                                mu_bc[:, :], OP.subtract)
                    # -var = mu^2 - E[x^2]
                    mup = prow.tile([1, QH], FP32, tag="mup", bufs=2,
                                    name=f"mup{which}{hh}")
                    nc.vector.tensor_tensor(mup[0:1, :], muf[0:1, :],
                                            muf[0:1, :], OP.mult)
                    nc.vector.tensor_tensor(mup[0:1, :], mup[0:1, :],
                                            msqf[0:1, :], OP.subtract)
                    # rstd = exp(-0.5 * ln(var + eps)); ACT Ln/Exp round trip
                    # measured at 1.1e-5 max rel on HW
                    rstdf = prow.tile([1, QH], BF16, tag="rstdf", bufs=2,
                                      name=f"rstdf{which}{hh}")
                    nc.scalar.activation(msqf[0:1, :], mup[0:1, :],
                                         AF.Ln, scale=-1.0,
                                         bias=bias_sb[0:1, EPS_COL:EPS_COL + 1])
                    nc.scalar.activation(rstdf[0:1, :], msqf[0:1, :],
                                         AF.Exp, scale=-0.5)
                    if pe_bcast:
                        rb_bc[hh] = rstdf
                    else:
                        rbc = prow.tile([128, QH], BF16, tag="rbc", bufs=2,
                                        name=f"rbc{which}{hh}")
                        nc.gpsimd.partition_broadcast(rbc[:, :], rstdf[0:1, :])
                        rb_bc[hh] = rbc
                for hh in halves:
                    sl = slice(hh * QH, (hh + 1) * QH)
                    if pe_bcast:
                        rb = pps.tile([128, QH], FP32, tag="avps", bufs=2,
                                      name=f"rb{which}{hh}")
                        nc.tensor.matmul(rb[:, :], ones_b[0:1, :],
                                         rb_bc[hh][0:1, :], start=True,
                                         stop=True)
                    else:
                        rb = rb_bc[hh]
                    for kt in range(DKT):
                        nt_seg = (x_tiles[kt] if in_place else out_tiles[kt])[:, sl]
                        nc.vector.tensor_tensor(nt_seg, nt_seg, rb[:, :],
                                                OP.mult)
                        if not ln_trivial[which - 1]:
                            nc.scalar.activation(nt_seg, nt_seg, AF.Identity,
                                                 bias=bcol(LN_B[which] + kt),
                                                 scale=bcol(LN_G[which] + kt))
                return out_tiles

            def head_proj(w_tiles, n_tiles, out_tiles, col_off, n_kt, tag,
                          hh_list=(0, 1)):
                """out^T[h][0:80, col_off:col_off+T] = w.T @ n, per-head padded.

                With both halves, they share one 2-bank PSUM tile and are
                evicted with a single copy. A single-half call (used to start
                consuming a half-finished LN) evicts just that half."""
                if len(hh_list) == 1:
                    hh = hh_list[0]
                    for h in range(H):
                        qp = pps.tile([128, 2 * QH], FP32, tag="sps", bufs=3,
                                      name=f"hp{tag}{h}{hh}")
                        for kt in range(n_kt):
                            nc.tensor.matmul(
                                qp[0:DH, 0:QH],
                                w_tiles[kt][:, h * DH:(h + 1) * DH],
                                n_tiles[kt][:, hh * QH:(hh + 1) * QH],
                                start=(kt == 0), stop=(kt == n_kt - 1))
                        nc.scalar.copy(
                            out_tiles[h][0:DH,
                                         col_off + hh * QH:col_off + (hh + 1) * QH],
                            qp[0:DH, 0:QH])
                    return
                for h in range(H):
                    qp = pps.tile([128, 2 * QH], FP32, tag="sps", bufs=3,
                                  name=f"hp{tag}{h}")
                    for hh in range(2):
                        for kt in range(n_kt):
                            nc.tensor.matmul(
                                qp[0:DH, hh * QH:(hh + 1) * QH],
                                w_tiles[kt][:, h * DH:(h + 1) * DH],
                                n_tiles[kt][:, hh * QH:(hh + 1) * QH],
                                start=(kt == 0), stop=(kt == n_kt - 1))
                    nc.scalar.copy(
                        out_tiles[h][0:DH, col_off:col_off + T], qp[0:DH, :])

            # V layout: per key-tile-pair blocks of 8 heads x (2 x 112)
            # cols: head h of slot kt lives at (kt//2)*1792 + h*224 +
            # (kt%2)*112, data cols 0:80, ones col 96. The j-pair stride of
            # 112B keeps dual-fp8 ldweights 16B-aligned AND contiguous so
            # the AV DoubleRow weight loads run at full speed.

            def v_heads(v_all, kt, half):
                base = (kt // 2) * 1792 + half * 896
                j = (kt % 2) * 112
                return v_all[:, base:base + 896].rearrange(
                    "p (h c) -> p h c", c=224)[:, :, j:j + 97]

            def v_proj(n_tiles, v_all, kt_slot, n_kt, w_tiles, n_tok, tok_off):
                """token-major V slot: data cols 0:80, ones col at 96 so the
                AV denominator lands on PSUM partition 96 (engine APs must
                start at partition 0/32/64/96)."""
                for half in range(2):
                    hv = v_heads(v_all, kt_slot, half)
                    nc.gpsimd.memset(hv[:, :, 80:96], 0.0)
                    nc.gpsimd.memset(hv[:, :, 96:97], 1.0)
                vpp = pps.tile([128, 2 * QH], FP32, tag="sps", bufs=3, name="vpp")
                for half in range(2):
                    vp = vpp[:, half * QH:half * QH + 320]
                    for kt in range(n_kt):
                        nc.tensor.matmul(
                            vp[0:n_tok, :],
                            n_tiles[kt][:, tok_off:tok_off + n_tok],
                            w_tiles[kt][:, half * 320:(half + 1) * 320],
                            start=(kt == 0), stop=(kt == n_kt - 1))
                    dst = v_heads(v_all, kt_slot, half)[0:n_tok, :, 0:80]
                    src = vp[0:n_tok, :].rearrange("p (h c) -> p h c", c=80)
                    nc.vector.tensor_copy(dst, src)

            def attention(qT_t, kT_t, v_all, n_keytiles, key_dim_last, aT_t,
                          e_pool, av_dr, pre_head=None, lag=2, xpool=None):
                """S^T -> exp -> AV; attention output is evicted unnormalized;
                each pair's denominator gets a fast approx reciprocal (single
                custom-DVE op, ~18 correct bits) straight off the AV PSUM ones
                column, and the normalize is emitted `lag` pairs late so the
                reciprocal latency hides behind the next pairs' matmuls.

                av_dr: E/V are fp8 and AV runs fp8 DoubleRow (2 key tiles
                per matmul at 0.5 cycles/row); exp gets EXP_BIAS so E stays
                under fp8 max normal (cancels in normalization)."""
                pend = {}
                drs = {}
                a_fp8 = aT_t[0].dtype == FP8
                e_dt = FP8 if av_dr else BF16

                def emit_normalize(p):
                    h, hh = p // 2, p % 2
                    rcb = drs.pop(p)
                    seg = aT_t[h][0:DH, hh * QH:(hh + 1) * QH]
                    src_seg = pend.pop(p)[0:DH, :] if a_fp8 else seg
                    nc.vector.tensor_tensor(seg, src_seg, rcb[0:DH, :],
                                            OP.mult)
                npairs = (n_keytiles + 1) // 2
                n_pairs = 2 * H

                def finish_pair(p, avp, evict_on_act=False):
                    """unnormalized evict + per-pair denominator chain.

                    evict_on_act alternates the PSUM reads between the scalar
                    engine (AF.Copy shares the exp table set, no reload) and
                    DVE per pair — cross-attn is throughput-bound on whichever
                    engine takes both, so split the load."""
                    h, hh = p // 2, p % 2
                    ev = nc.scalar.copy if evict_on_act else nc.vector.tensor_copy
                    ev2 = nc.vector.tensor_copy
                    if a_fp8:
                        ub = prcb.tile([128, QH], BF16, tag="ub", bufs=3,
                                       name=f"ub{h}{hh}")
                        pend[p] = ub
                        ev(ub[0:DH, :], avp[0:DH, :])
                    else:
                        ev(aT_t[h][0:DH, hh * QH:(hh + 1) * QH], avp[0:DH, :])
                    # den to SBUF (the custom-DVE recip misreads PSUM on
                    # HW), fast fp32 reciprocal on DVE, then a gpsimd
                    # partition_broadcast into SBUF — the PE never touches
                    # the denominator chain
                    dnf = prcb.tile([1, QH], FP32, tag="dnf", bufs=2,
                                    name=f"dnf{h}{hh}")
                    ev2(dnf[0:1, :], avp[96:97, :])
                    drf = prcb.tile([1, QH], FP32, tag="drf", bufs=2,
                                    name=f"drf{h}{hh}")
                    nc.vector.reciprocal_approx_fast(drf[0:1, :], dnf[0:1, :])
                    rcb = prcb.tile([DH, QH], FP32, tag="dr", bufs=lag + 1,
                                    name=f"dr{h}{hh}")
                    nc.gpsimd.partition_broadcast(rcb[:, :], drf[0:1, :])
                    drs[p] = rcb
                    if p >= lag:
                        emit_normalize(p - lag)

                if npairs == 1 and not av_dr:
                    # single key tile (cross-attn): pipeline scores/exp one
                    # PAIR ahead of AV so the PE never waits on the exp
                    klen = key_dim_last
                    sc = {}
                    for idx in range(n_pairs + 1):
                        if idx < n_pairs:
                            h, hh = idx // 2, idx % 2
                            spp = pps.tile([128, 2 * QH], FP32, tag="sps",
                                           bufs=3, name=f"s{h}{hh}")
                            nc.tensor.matmul(
                                spp[0:klen, 0:QH], kT_t[h][0:DH, 0:klen],
                                qT_t[h][0:DH, hh * QH:(hh + 1) * QH],
                                start=True, stop=True)
                            et = e_pool.tile([128, 2 * QH], e_dt, tag="E",
                                             name=f"e{h}{hh}")
                            nc.scalar.activation(et[0:klen, 0:QH],
                                                 spp[0:klen, 0:QH],
                                                 AF.Exp, scale=scale)
                            sc[idx] = et
                        if idx > 0:
                            p = idx - 1
                            h, hh = p // 2, p % 2
                            pet = sc.pop(p)
                            avp = pps.tile([128, 2 * QH], FP32, tag="sps",
                                           bufs=3, name=f"av{h}{hh}")[:, 0:QH]
                            vh = v_heads(v_all, 0, h // 4)
                            nc.tensor.matmul(avp[0:97, :],
                                             vh[0:klen, h % 4, :],
                                             pet[0:klen, 0:QH],
                                             start=True, stop=True)
                            finish_pair(p, avp, evict_on_act=True)
                    for p in sorted(drs):
                        emit_normalize(p)
                    return

                for h in range(H):
                    if pre_head is not None:
                        pre_head(h)
                    at = aT_t[h]
                    for hh in range(2):
                        p = h * 2 + hh
                        avp = pps.tile([128, QH], FP32, tag="avps", bufs=2,
                                       name=f"av{h}{hh}")
                        # two score tiles share one 2-bank PSUM tile so a
                        # single exp covers both; pipelined one pair ahead of
                        # the AV consumers
                        ets = {}
                        for pt in range(npairs + 1):
                            if pt < npairs:
                                kts = [kt for kt in (2 * pt, 2 * pt + 1)
                                       if kt < n_keytiles]
                                spp = pps.tile([128, 2 * QH], FP32, tag="sps",
                                               bufs=3, name=f"s{h}{hh}{pt}")
                                klens = []
                                for j, kt in enumerate(kts):
                                    klen = (key_dim_last
                                            if kt == n_keytiles - 1 else 128)
                                    klens.append(klen)
                                    nc.tensor.matmul(
                                        spp[0:klen, j * QH:(j + 1) * QH],
                                        kT_t[h][0:DH, kt * 128:kt * 128 + klen],
                                        qT_t[h][0:DH, hh * QH:(hh + 1) * QH],
                                        start=True, stop=True)
                                et = e_pool.tile([128, 2 * QH], e_dt, tag="E",
                                                 name=f"e{h}{hh}{pt}")
                                eb = (dict(bias=bias_sb[0:128, EXPB_COL:EXPB_COL + 1])
                                      if av_dr else {})
                                if len(kts) == 2 and klens[0] == klens[1]:
                                    nc.scalar.activation(
                                        et[0:klens[0], :], spp[0:klens[0], :],
                                        AF.Exp, scale=scale, **eb)
                                else:
                                    for j, kt in enumerate(kts):
                                        nc.scalar.activation(
                                            et[0:klens[j], j * QH:(j + 1) * QH],
                                            spp[0:klens[j], j * QH:(j + 1) * QH],
                                            AF.Exp, scale=scale, **eb)
                                ets[pt] = (et, kts, klens)
                            if pt > 0:
                                pet, pkts, pklens = ets.pop(pt - 1)
                                vb = (pt - 1) * 1792 + h * 224
                                v_ap = v_all[:, vb:vb + 224].rearrange(
                                    "p (j c) -> p j c", c=112)[:, :, 0:97]
                                e_ap = pet[:].rearrange(
                                    "p (j f) -> p j f", f=QH)
                                nc.tensor.matmul(
                                    avp[0:97, :], v_ap, e_ap,
                                    start=(pt == 1), stop=(pt == npairs),
                                    perf_mode=DR)
                        # fp8 aT can't hold unnormalized values (they exceed
                        # 240): finish_pair parks them in bf16 and the lagged
                        # normalize writes the fp8 tile
                        finish_pair(p, avp)
                for p in sorted(drs):
                    emit_normalize(p)

            def out_proj(wp_ap, aT_pairs, res_t, bias_off, hh):
                """res[:, half hh] += aT @ o^T + bias (in-place update).

                Emitted one query-half at a time so the following LN's
                stats can interleave with the other half's matmuls.
                USE_O_DR: fp8 DoubleRow over head pairs (contraction subtile
                = head; weight rows 80:128 of each head slab are zero, which
                also kills the untouched aT padding rows)."""
                sl = slice(hh * QH, (hh + 1) * QH)
                for m in range(DKT):
                    op_ = pps.tile([128, 2 * QH], FP32, tag="sps", bufs=3,
                                   name=f"op{m}{hh}")
                    # preload the residual into PSUM with an identity matmul
                    # so the update needs no DVE pass: the scalar engine
                    # evicts PSUM + bias straight back to res
                    nc.tensor.matmul(op_[:, 0:QH], ident_sb[:, :],
                                     res_t[m][:, sl], start=True, stop=False)
                    if USE_O_DR:
                        for j in range(H // 2):
                            w_ap = wp_ap.rearrange(
                                "p (h c) -> p h c", c=D)[
                                :, 2 * j:2 * j + 2, m * 128:(m + 1) * 128]
                            a_ap = aT_pairs[j].rearrange(
                                "p (h c) -> p h c", c=T)[
                                :, 0:2, sl]
                            nc.tensor.matmul(
                                op_[:, 0:QH], w_ap, a_ap,
                                start=False, stop=(j == H // 2 - 1),
                                perf_mode=DR)
                    else:
                        for kt in range(H):
                            nc.tensor.matmul(
                                op_[:, 0:QH],
                                wp_ap.rearrange("p (h c) -> p h c", c=D)[
                                    :, kt, m * 128:(m + 1) * 128],
                                aT_pairs[kt // 2].rearrange(
                                    "p (h c) -> p h c", c=T)[
                                    :, kt % 2, sl],
                                start=False, stop=(kt == H - 1))
                    nc.scalar.activation(res_t[m][:, sl], op_[:, 0:QH],
                                         AF.Identity,
                                         bias=bcol(bias_off + m))

            # residual stream (feature-major, f32)
            res_tiles = []
            for kt in range(DKT):
                rt = pres.tile([128, T], F32R, tag="res", name=f"res_{kt}")
                res_tiles.append(rt)
            # split by column half, all hh=0 halves first, so the hh=0 LN
            # stats chain starts after half the frame has landed
            for hh in range(2):
                for kt in range(DKT):
                    nc.sync.dma_start(
                        res_tiles[kt][:, hh * QH:(hh + 1) * QH],
                        dram["hsT_q"][kt * 128:(kt + 1) * 128,
                                      hh * QH:(hh + 1) * QH])
            # constants ride behind the residual halves (first needed ~15us in)
            nc.sync.dma_start(bias_sb[:], dram["biases"][:])
            nc.sync.dma_start(ident_sb[:], dram["ident"][:])

            a_dt = FP8 if USE_O_DR else BF16

            with ExitStack() as ctx_abcd:
                pqT = ctx_abcd.enter_context(tc.tile_pool(name="qT", bufs=11))
                paT = ctx_abcd.enter_context(tc.tile_pool(name="aT", bufs=8))

                def alloc_aT(nm):
                    """per-head-pair aT tiles: out_proj's pair j then depends
                    only on its own pairs' normalizes, not the whole flush"""
                    pairs = [paT.tile([128, 2 * T], a_dt, tag="aT",
                                      name=f"{nm}_{j}") for j in range(4)]
                    for pt in pairs:
                        nc.gpsimd.memset(
                            pt[64:128, :].bitcast(FP32)
                            if a_dt == FP8 else pt[64:128, :], 0.0)
                    tiles = [pairs[h // 2][:, (h % 2) * T:(h % 2 + 1) * T]
                             for h in range(H)]
                    return pairs, tiles
                penc = ctx_abcd.enter_context(tc.tile_pool(name="enc", bufs=6))
                pk2 = ctx_abcd.enter_context(tc.tile_pool(name="k2T", bufs=8))
                pV2 = ctx_abcd.enter_context(tc.tile_pool(name="V2", bufs=1))

                # ---------- phase A: LN1 + QKV projections ----------
                with ExitStack() as ctx_b:
                    pkT = ctx_b.enter_context(tc.tile_pool(name="kT", bufs=8))
                    pV = ctx_b.enter_context(tc.tile_pool(name="V", bufs=1))
                    pE = ctx_b.enter_context(tc.tile_pool(name="E", bufs=5))

                    kT_tiles = [pkT.tile([128, 2 * T], BF16, tag="kT", name=f"kT_{h}")
                                for h in range(H)]
                    v_all = pV.tile([128, TT * 1792], FP8, tag="V", name="v_all")

                    fr0_tiles = []
                    fr1_tiles = []
                    for kt in range(DKT):
                        ft = pn.tile([128, T], BF16, tag="fr", bufs=10,
                                     name=f"fr0_{kt}")
                        nc.sync.dma_start(
                            ft[:], dram["hsT_first"][kt * 128:(kt + 1) * 128, :])
                        fr0_tiles.append(ft)
                        ft1 = pn.tile([128, T], BF16, tag="fr", bufs=10,
                                      name=f"fr1_{kt}")
                        nc.sync.dma_start(
                            ft1[:], dram["hsT_former"][kt * 128:(kt + 1) * 128, :])
                        fr1_tiles.append(ft1)
                    n_q = emit_ln(res_tiles, 1, [], pe_bcast=True,
                                  sq_on_dve=True)
                    emit_ln(fr0_tiles, 1, fr0_tiles,  # in place, overlaps Q
                            sq_on_dve=True)
                    q1_sb = load_w("q1", DKT, "w", pw)
                    qT_tiles = [pqT.tile([128, T], BF16, tag="qT", name=f"qT_{h}")
                                for h in range(H)]
                    head_proj(q1_sb, n_q, qT_tiles, 0, DKT, "q")

                    # cross-attention K/V from the text context have no
                    # dependence on the residual stream: emit them here so
                    # their PE/DVE work fills phase A/B stalls and phase D
                    # starts with K2/V2 ready.
                    enc_tiles = []
                    for kt in range(CKT):
                        et_ = penc.tile([128, CTXP], BF16, tag="enc", name=f"enc_{kt}")
                        nc.sync.dma_start(
                            et_[:], dram["encT"][kt * 128:(kt + 1) * 128, :])
                        enc_tiles.append(et_)
                    k2_sb = load_w("k2", CKT, "w", pw)
                    k2T_tiles = [pk2.tile([128, CTXP], BF16, tag="k2T", name=f"k2T_{h}")
                                 for h in range(H)]
                    for h in range(H):
                        kp = pps.tile([128, QH], FP32, tag="avps", bufs=2,
                                      name=f"k2p{h}")
                        for kt in range(CKT):
                            nc.tensor.matmul(kp[0:DH, 0:CTXP],
                                             k2_sb[kt][:, h * DH:(h + 1) * DH],
                                             enc_tiles[kt][:],
                                             start=(kt == 0), stop=(kt == CKT - 1))
                        nc.vector.tensor_copy(k2T_tiles[h][0:DH, :], kp[0:DH, 0:CTXP])
                    v2_sb = load_w("v2", CKT, "w", pw)
                    v2_all = pV2.tile([128, 1792], BF16, tag="V2", name="v2t")
                    v_proj(enc_tiles, v2_all, 0, CKT, v2_sb, CTX, 0)

                    emit_ln(fr1_tiles, 1, fr1_tiles, sq_on_dve=True)  # in place
                    k1_sb = load_w("k1", DKT, "w", pw)
                    frames = (fr0_tiles, fr1_tiles)
                    for fi, fr_tiles in enumerate(frames):
                        head_proj(k1_sb, fr_tiles, kT_tiles, fi * T, DKT, f"k{fi}")
                    v1_sb = load_w("v1", DKT, "w", pw)
                    for fi, fr_tiles in enumerate(frames):
                        for tt in range(TT):
                            v_proj(fr_tiles, v_all, fi * TT + tt, DKT, v1_sb,
                                   128, tt * 128)

                    # ---------- phase B: sparse-causal attention ----------
                    aT_pairs, aT_tiles = alloc_aT("aT")
                    attention(qT_tiles, kT_tiles, v_all, 2 * TT, 128, aT_tiles,
                              pE, av_dr=True)

                # ---------- phase C/D: o1 + LN2 interleaved by half, then
                # cross attention, then o2 + LN3 interleaved by half ----------
                with ExitStack() as ctx_d:
                    pwp = ctx_d.enter_context(tc.tile_pool(name="wp", bufs=1))
                    o1p_sb = pwp.tile([128, H * D], a_dt, tag="wp", name="o1p_sb")
                    # split by head-pair slab so the first DR matmul starts
                    # after a quarter of the weights have landed
                    for j in range(4):
                        nc.sync.dma_start(
                            o1p_sb[:, j * 2 * D:(j + 1) * 2 * D],
                            dram["o1p"][:, j * 2 * D:(j + 1) * 2 * D])
                    pE2 = ctx_d.enter_context(tc.tile_pool(name="E2", bufs=5))
                    pxat = ctx_d.enter_context(tc.tile_pool(name="xat", bufs=2))
                    pwp2 = ctx_d.enter_context(tc.tile_pool(name="wp2", bufs=1))
                    # o2 weights fetched early so the DMA overlaps
                    # LN2/q2/attention instead of stalling the o2 seam
                    o2p_sb = pwp2.tile([128, H * D], a_dt, tag="wp2", name="o2p_sb")
                    nc.sync.dma_start(o2p_sb[:], dram["o2p"][:])
                    q2_sb = load_w("q2", DKT, "w", pw)

                    # o1 half h feeds LN2 half h stats while the PE runs the
                    # other half's matmuls — the LN chain latency hides
                    n2 = []
                    out_proj(o1p_sb[:], aT_pairs, res_tiles, OB1, 0)
                    emit_ln(res_tiles, 2, n2, halves=(0,))
                    out_proj(o1p_sb[:], aT_pairs, res_tiles, OB1, 1)
                    emit_ln(res_tiles, 2, n2, halves=(1,))
                    q2T_tiles = [pqT.tile([128, T], BF16, tag="qT", name=f"q2T_{h}")
                                 for h in range(H)]
                    head_proj(q2_sb, n2, q2T_tiles, 0, DKT, "q2", hh_list=(0,))
                    head_proj(q2_sb, n2, q2T_tiles, 0, DKT, "q2", hh_list=(1,))

                    a2T_pairs, a2T_tiles = alloc_aT("a2T")
                    attention(q2T_tiles, k2T_tiles, v2_all, 1, CTX, a2T_tiles,
                              pE2, av_dr=False, xpool=pxat)
                    n3 = []
                    out_proj(o2p_sb[:], a2T_pairs, res_tiles, OB2, 0)
                    emit_ln(res_tiles, 3, n3, halves=(0,))
                    out_proj(o2p_sb[:], a2T_pairs, res_tiles, OB2, 1)
                    emit_ln(res_tiles, 3, n3, halves=(1,))

            # ---------- phase E: GEGLU feed-forward ----------
            with ExitStack() as ctx_e:
                pG = ctx_e.enter_context(tc.tile_pool(name="gT", bufs=20))
                pgl = ctx_e.enter_context(tc.tile_pool(name="gl", bufs=3))
                pff2 = ctx_e.enter_context(tc.tile_pool(name="ff2w", bufs=20))

                gT_tiles = []
                fxg = {}

                def ff1_tile(mi):
                    fx = pw.tile([128, D], BF16, tag="w", name=f"fx{mi}")
                    fg = pw.tile([128, D], BF16, tag="w", name=f"fg{mi}")
                    fx_dst = fx[:].rearrange("p (k c) -> p k c", c=128)
                    fg_dst = fg[:].rearrange("p (k c) -> p k c", c=128)
                    fx_src = dram["ff1b"][mi].rearrange("(k p) c -> p k c", p=128)
                    fg_src = dram["ff1b"][FMT + mi].rearrange("(k p) c -> p k c", p=128)
                    nc.sync.dma_start(fx_dst, fx_src)
                    nc.sync.dma_start(fg_dst, fg_src)
                    fxg[mi] = (fx, fg)
                    gt = pG.tile([128, T], BF16, tag="gT", name=f"gT_{mi}")
                    gT_tiles.append(gt)

                # the first tiles run their hh=0 half only, so LN3's hh=1
                # chain drains while the PE is already projecting
                STAG = 5
                seq = [(mi, 0) for mi in range(STAG)]
                seq += [(mi, 1) for mi in range(STAG)]
                seq += [(mi, hh) for mi in range(STAG, FMT) for hh in range(2)]
                for mi, hh in seq:
                    if hh == 0:
                        if mi not in fxg:
                            ff1_tile(mi)
                    if True:
                        fx, fg = fxg[mi]
                        gt = gT_tiles[mi]
                        xgp = pps.tile([128, 2 * QH], FP32, tag="sps", bufs=3,
                                       name=f"xgp{mi}{hh}")
                        xp = xgp[:, 0:QH]
                        gp = xgp[:, QH:2 * QH]
                        for kt in range(DKT):
                            nc.tensor.matmul(
                                xp[:, :], fx[:, kt * 128:(kt + 1) * 128],
                                n3[kt][:, hh * QH:(hh + 1) * QH],
                                start=(kt == 0), stop=(kt == DKT - 1))
                        for kt in range(DKT):
                            nc.tensor.matmul(
                                gp[:, :], fg[:, kt * 128:(kt + 1) * 128],
                                n3[kt][:, hh * QH:(hh + 1) * QH],
                                start=(kt == 0), stop=(kt == DKT - 1))
                        gl = pgl.tile([128, QH], BF16, tag="gl", name=f"gl{mi}{hh}")
                        nc.scalar.activation(gl[:], gp[:, :], GELU_AF or AF.Gelu,
                                             bias=bcol(FBG + mi), scale=1.0)
                        nc.vector.scalar_tensor_tensor(
                            gt[:, hh * QH:(hh + 1) * QH], xp[:, :], bcol(FBX + mi),
                            gl[:], OP.add, OP.mult)

                ff2_sb = load_w("ff2", FMT, "ff2w", pff2, dtype=BF16)
                for hh in range(2):
                    sl = slice(hh * QH, (hh + 1) * QH)
                    for m in range(DKT):
                        fp = pps.tile([128, 2 * QH], FP32, tag="sps", bufs=3,
                                      name=f"fp{m}{hh}")
                        if hh == 0:
                            nc.tensor.matmul(fp[:, 0:QH], ident_sb[:, :],
                                             res_tiles[m][:, sl], start=True,
                                             stop=False)
                        for kt in range(FMT):
                            nc.tensor.matmul(
                                fp[:, 0:QH],
                                ff2_sb[kt][:, m * 128:(m + 1) * 128],
                                gT_tiles[kt][:, sl],
                                start=(hh == 1 and kt == 0),
                                stop=(kt == FMT - 1))
                        if hh == 0:
                            nc.scalar.activation(res_tiles[m][:, sl],
                                                 fp[:, 0:QH], AF.Identity,
                                                 bias=bcol(FB2 + m))
                        else:
                            # second half drains on DVE (idle at the end) so
                            # ACT and DVE evict the tail in parallel
                            nc.vector.scalar_tensor_tensor(
                                res_tiles[m][:, sl], fp[:, 0:QH],
                                bcol(FB2 + m), res_tiles[m][:, sl],
                                OP.add, OP.add)
                        nc.sync.dma_start(
                            out_dram[m * 128:(m + 1) * 128, sl],
                            res_tiles[m][:, sl])

    nc.compile()
    return nc


def _install_ntff_shim():
    """Register the axon NTFF profile hook (profiling only; this container's
    antenv lacks the axon_hooks shim module)."""
    import types
    if "antenv.axon_hooks" in sys.modules:
        return
    mod = types.ModuleType("antenv.axon_hooks")
    mod._hook = None
    mod.set_axon_ntff_profile_hook = lambda h: setattr(mod, "_hook", h)
    mod.get_axon_ntff_profile_hook = lambda: mod._hook
    sys.modules["antenv.axon_hooks"] = mod
    try:
        from trn_agent_boot.trn_boot import _ntff_profile_via_ctypes
        mod._hook = _ntff_profile_via_ctypes("/opt/axon/libaxon_pjrt.so")
    except Exception:
        pass


_PROGRAM_CACHE = {}


def _get_program(ln_trivial):
    key = (tuple(ln_trivial), GELU_AF)
    if key not in _PROGRAM_CACHE:
        _PROGRAM_CACHE[key] = build_program(ln_trivial)
    return _PROGRAM_CACHE[key]


def _pad_heads(w):
    """[640, 640] head rows -> [128, 8*640] per-head slabs (rows 0:80)."""
    out = np.zeros((128, H * D), np.float32)
    for h in range(H):
        out[:DH, h * D:(h + 1) * D] = w[h * DH:(h + 1) * DH]
    return out


def _bias_cols(vec, n):
    return np.ascontiguousarray(vec.reshape(n, 128).T)


def kernel(**inputs):
    hs = np.ascontiguousarray(inputs["hidden_states"], np.float32)
    enc = np.ascontiguousarray(inputs["encoder_hidden_states"], np.float32)
    f = int(inputs["video_length"])
    BF = hs.shape[0]
    assert BF == N_CORES and hs.shape[1:] == (T, D)

    ln_trivial = tuple(
        bool(np.all(inputs[f"n{i}_g"] == 1.0) and np.all(inputs[f"n{i}_b"] == 0.0))
        for i in (1, 2, 3))
    nc = _get_program(ln_trivial)

    biases = np.zeros((128, NB), np.float32)
    biases[:, EPS_COL] = LN_EPS
    biases[:, EXPB_COL] = EXP_BIAS
    biases[:, OB1:OB1 + 5] = _bias_cols(inputs["o1_b"].astype(np.float32), 5)
    biases[:, OB2:OB2 + 5] = _bias_cols(inputs["o2_b"].astype(np.float32), 5)
    biases[:, FB2:FB2 + 5] = _bias_cols(inputs["ff2_b"].astype(np.float32), 5)
    ff1_b = inputs["ff1_b"].astype(np.float32)
    biases[:, FBX:FBX + FMT] = _bias_cols(ff1_b[:DFF], FMT)
    biases[:, FBG:FBG + FMT] = _bias_cols(ff1_b[DFF:], FMT)
    for i in (1, 2, 3):
        biases[:, LN_G[i]:LN_G[i] + 5] = _bias_cols(inputs[f"n{i}_g"].astype(np.float32), 5)
        biases[:, LN_B[i]:LN_B[i] + 5] = _bias_cols(inputs[f"n{i}_b"].astype(np.float32), 5)

    ff1 = inputs["ff1"].astype(np.float32)  # [640, 5120]
    ff1b = np.ascontiguousarray(
        ff1.reshape(DKT, 128, 2 * FMT, 128).transpose(2, 0, 1, 3)
        .reshape(2 * FMT, D, 128)).astype(ml_dtypes.bfloat16)

    o_np = E4M3 if USE_O_DR else ml_dtypes.bfloat16
    common = {
        "q1": inputs["q1"].astype(ml_dtypes.bfloat16),
        "k1": inputs["k1"].astype(ml_dtypes.bfloat16),
        "v1": inputs["v1"].astype(ml_dtypes.bfloat16),
        "q2": inputs["q2"].astype(ml_dtypes.bfloat16),
        "k2": inputs["k2"].astype(ml_dtypes.bfloat16),
        "v2": inputs["v2"].astype(ml_dtypes.bfloat16),
        "o1p": _pad_heads(inputs["o1"].astype(np.float32)).astype(o_np),
        "o2p": _pad_heads(inputs["o2"].astype(np.float32)).astype(o_np),
        "ff1b": ff1b,
        "ff2": np.ascontiguousarray(inputs["ff2"], np.float32).astype(ml_dtypes.bfloat16),
        "biases": biases,
        "ident": np.eye(128, dtype=np.float32),
    }

    hsT = np.ascontiguousarray(hs.transpose(0, 2, 1))      # [BF, 640, 1024]
    hsT_bf = hsT.astype(ml_dtypes.bfloat16)
    encT = np.zeros((BF, CROSS, CTXP), np.float32)         # ctx padded 77 -> 80
    encT[:, :, :CTX] = enc.transpose(0, 2, 1)
    encT_bf = encT.astype(ml_dtypes.bfloat16)
    in_maps = []
    for g in range(BF):
        bi, fi = divmod(g, f)
        first = bi * f
        former = bi * f + max(fi - 1, 0)
        in_maps.append({
            **common,
            "hsT_q": hsT[g],
            "hsT_first": hsT_bf[first],
            "hsT_former": hsT_bf[former],
            "encT": encT_bf[g],
        })

    want_trace = bool(int(os.environ.get("KERNEL_TRACE", "0")))
    if want_trace:
        _install_ntff_shim()
    res = run_bass_kernel_spmd(nc, in_maps, core_ids=list(range(N_CORES)),
                               trace=want_trace)
    kernel.last_results = res
    out = np.stack([res.results[g]["outT"].T for g in range(BF)])
    return np.ascontiguousarray(out.astype(inputs["hidden_states"].dtype))

